# revision 1
# baseline (speedup 1.0000x reference)
"""Paged-attention GPT-2 decode kernel for 8 Trainium2 NeuronCores.

Sharding: tensor-parallel across heads (Megatron) — 2 heads per core.
Each core gets its head-pair slice of w_attn / w_proj / KV caches and
computes a partial [32,1024] c_proj output; host sums the 8 partials.

The program is specialized to the observed context_lens/block_tables
(static DMA descriptors reading exactly ctx-1 cached tokens per seq;
the new token's K/V is computed on-device and folded in from SBUF).
Compiled programs are cached by a hash of those tensors.
"""

import numpy as np

NUM_SEQS = 32
EMBED = 1024
NUM_HEADS = 16
HEAD_DIM = 64
BLOCK_SIZE = 16
N_CORES = 8
HEADS_PER_CORE = NUM_HEADS // N_CORES          # 2
HD = HEADS_PER_CORE * HEAD_DIM                 # 128
SCALE = HEAD_DIM ** -0.5
TOT_SLOTS = 8192 * BLOCK_SIZE                  # 131072
KDIM = EMBED + 1                               # augmented contraction (bias row)
P = 128


def _runs_for_seq(bt_row, n_tok):
    """Coalesce the first n_tok token slots of a sequence into contiguous runs."""
    runs = []
    for t in range(0, n_tok, BLOCK_SIZE):
        blk = int(bt_row[t // BLOCK_SIZE])
        start = blk * BLOCK_SIZE
        cnt = min(BLOCK_SIZE, n_tok - t)
        if runs and runs[-1][0] + runs[-1][1] == start:
            runs[-1][1] += cnt
        else:
            runs.append([start, cnt])
    return runs


def _subtiles_for_seq(bt_row, n_tok):
    """Split a seq's cached tokens into <=128-token pieces, each one contiguous slot run."""
    out = []
    for start, cnt in _runs_for_seq(bt_row, n_tok):
        off = 0
        while off < cnt:
            n = min(P, cnt - off)
            out.append((start + off, n))
            off += n
    return out


def _build_program(context_lens, block_tables):
    import concourse.bass as bass
    import concourse.bacc as bacc
    import concourse.tile as tile
    from concourse import mybir

    fp32 = mybir.dt.float32
    nc = bacc.Bacc("TRN2", target_bir_lowering=False)

    hT = nc.declare_dram_parameter("hT", [KDIM, NUM_SEQS], fp32, isOutput=False)
    wqkv = nc.declare_dram_parameter("wqkv", [KDIM, 3 * HD], fp32, isOutput=False)
    wproj = nc.declare_dram_parameter("wproj", [HD, EMBED], fp32, isOutput=False)
    kc = nc.declare_dram_parameter("kc", [TOT_SLOTS, HD], fp32, isOutput=False)
    vc = nc.declare_dram_parameter("vc", [TOT_SLOTS, HD], fp32, isOutput=False)
    out_part = nc.declare_dram_parameter("out_part", [NUM_SEQS, EMBED], fp32, isOutput=True)
    qkv_dram = nc.dram_tensor("qkv_scratch", [NUM_SEQS, 3 * HD], fp32)

    with tile.TileContext(nc) as tc:
        with (
            tc.tile_pool(name="persist", bufs=1) as persist,
            tc.tile_pool(name="kvp", bufs=8) as kvp,
            tc.tile_pool(name="small", bufs=4) as small,
            tc.tile_pool(name="psum", bufs=1, space="PSUM") as psum,
        ):
            # ---- constants / weights into SBUF ----
            ones_row = persist.tile([1, P], fp32)
            nc.vector.memset(ones_row, 1.0)
            ones_col = persist.tile([P, 1], fp32)
            nc.vector.memset(ones_col, 1.0)

            hT_sb = persist.tile([P, 9, NUM_SEQS], fp32)
            w_sb = persist.tile([P, 9, 3 * HD], fp32)
            for i in range(8):
                nc.sync.dma_start(out=hT_sb[:, i, :], in_=hT[i * P:(i + 1) * P, :])
                nc.sync.dma_start(out=w_sb[:, i, :], in_=wqkv[i * P:(i + 1) * P, :])
            nc.sync.dma_start(out=hT_sb[:1, 8, :], in_=hT[EMBED:KDIM, :])
            nc.sync.dma_start(out=w_sb[:1, 8, :], in_=wqkv[EMBED:KDIM, :])
            wproj_sb = persist.tile([HD, EMBED], fp32)
            nc.sync.dma_start(out=wproj_sb, in_=wproj[:, :])

            # ---- qkv projection: [32, 384] = hidden_aug @ w_aug ----
            qkv_ps = psum.tile([NUM_SEQS, 3 * HD], fp32, tag="qkv", bufs=1)
            for i in range(9):
                pp = P if i < 8 else 1
                nc.tensor.matmul(
                    qkv_ps,
                    lhsT=hT_sb[:pp, i, :],
                    rhs=w_sb[:pp, i, :],
                    start=(i == 0),
                    stop=(i == 8),
                )
            qkv_sb = persist.tile([NUM_SEQS, 3 * HD], fp32)
            # fold the attention scale into q here
            nc.scalar.mul(qkv_sb[:, 0:HD], qkv_ps[:, 0:HD], SCALE)
            nc.scalar.copy(qkv_sb[:, HD:3 * HD], qkv_ps[:, HD:3 * HD])
            # bounce through DRAM so per-seq rows can be reloaded at partition 0
            nc.sync.dma_start(out=qkv_dram[:, :], in_=qkv_sb)

            ctxT_all = persist.tile([P, NUM_SEQS], fp32)

            # ---- per-sequence attention ----
            for s in range(NUM_SEQS):
                ctx_len = int(context_lens[s])
                n_cache = ctx_len - 1
                subtiles = _subtiles_for_seq(block_tables[s], n_cache)

                # reload this seq's qkv row at partition 0 (PE/engine alignment rules)
                qkvrow = small.tile([1, 3 * HD], fp32, tag="qkvrow", bufs=2)
                nc.sync.dma_start(out=qkvrow, in_=qkv_dram[s:s + 1, :])
                qrow = qkvrow[:, 0:HD]
                krow = qkvrow[:, HD:2 * HD]
                vrow = qkvrow[:, 2 * HD:3 * HD]
                # broadcast this seq's (scaled) q to all 128 partitions
                qb_ps = psum.tile([P, P], fp32, tag="qb", bufs=2)
                nc.tensor.matmul(qb_ps, lhsT=ones_row, rhs=qrow,
                                 start=True, stop=True)
                qb = small.tile([P, P], fp32, tag="qb_sb", bufs=2)
                nc.vector.tensor_copy(qb, qb_ps)
                qb3 = qb.rearrange("p (h d) -> p h d", h=HEADS_PER_CORE)

                ctx_ps = psum.tile([P, HEADS_PER_CORE], fp32, tag="ctx", bufs=2)
                sum_ps = psum.tile([1, HEADS_PER_CORE], fp32, tag="sums", bufs=1)
                first = True
                for slot0, n in subtiles:
                    kt = kvp.tile([P, HD], fp32, tag="k")
                    vt = kvp.tile([P, HD], fp32, tag="v")
                    nc.sync.dma_start(out=kt[:n, :], in_=kc[slot0:slot0 + n, :])
                    nc.sync.dma_start(out=vt[:n, :], in_=vc[slot0:slot0 + n, :])
                    tmp = small.tile([P, HEADS_PER_CORE, HEAD_DIM], fp32, tag="tmp")
                    nc.vector.tensor_mul(
                        tmp[:n], kt[:n, :].rearrange("p (h d) -> p h d", h=HEADS_PER_CORE),
                        qb3[:n])
                    sc = small.tile([P, HEADS_PER_CORE], fp32, tag="sc")
                    nc.vector.reduce_sum(sc[:n], tmp[:n], axis=mybir.AxisListType.X)
                    pr = small.tile([P, HEADS_PER_CORE], fp32, tag="pr")
                    nc.scalar.activation(pr[:n], sc[:n], mybir.ActivationFunctionType.Exp)
                    nc.tensor.matmul(ctx_ps, lhsT=vt[:n, :], rhs=pr[:n],
                                     start=first, stop=False)
                    nc.tensor.matmul(sum_ps, lhsT=ones_col[:n, :],
                                     rhs=pr[:n], start=first, stop=False)
                    first = False

                # new token (position ctx_len-1): K/V straight from SBUF
                tmp_t = small.tile([1, HEADS_PER_CORE, HEAD_DIM], fp32, tag="tmpt", bufs=2)
                nc.vector.tensor_mul(
                    tmp_t,
                    krow.rearrange("p (h d) -> p h d", h=HEADS_PER_CORE),
                    qb3[:1])
                sc_t = small.tile([1, HEADS_PER_CORE], fp32, tag="sct", bufs=2)
                nc.vector.reduce_sum(sc_t, tmp_t, axis=mybir.AxisListType.X)
                pr_t = small.tile([1, HEADS_PER_CORE], fp32, tag="prt", bufs=2)
                nc.scalar.activation(pr_t, sc_t, mybir.ActivationFunctionType.Exp)
                nc.tensor.matmul(ctx_ps, lhsT=vrow, rhs=pr_t,
                                 start=first, stop=True)
                nc.tensor.matmul(sum_ps, lhsT=ones_col[:1, :],
                                 rhs=pr_t, start=first, stop=True)

                # normalize: ctx / sum  (broadcast 1/sum to all partitions via PE)
                sums_sb = small.tile([1, HEADS_PER_CORE], fp32, tag="ssb", bufs=2)
                nc.vector.tensor_copy(sums_sb, sum_ps)
                rs = small.tile([1, HEADS_PER_CORE], fp32, tag="rs", bufs=2)
                nc.vector.reciprocal(rs, sums_sb)
                rsb_ps = psum.tile([P, HEADS_PER_CORE], fp32, tag="rsb", bufs=1)
                nc.tensor.matmul(rsb_ps, lhsT=ones_row, rhs=rs, start=True, stop=True)
                rsb_sb = small.tile([P, HEADS_PER_CORE], fp32, tag="rsbsb", bufs=2)
                nc.vector.tensor_copy(rsb_sb, rsb_ps)
                scaled = small.tile([P, HEADS_PER_CORE], fp32, tag="scaled", bufs=2)
                nc.vector.tensor_mul(scaled, ctx_ps, rsb_sb)
                for h in range(HEADS_PER_CORE):
                    nc.vector.tensor_copy(
                        ctxT_all[h * HEAD_DIM:(h + 1) * HEAD_DIM, s:s + 1],
                        scaled[h * HEAD_DIM:(h + 1) * HEAD_DIM, h:h + 1])

            # ---- c_proj partial: [32, 1024] = ctxT.T @ wproj_slice ----
            out_sb = persist.tile([NUM_SEQS, EMBED], fp32)
            for nblk in range(2):
                cp_ps = psum.tile([NUM_SEQS, 512], fp32, tag="cp", bufs=1)
                nc.tensor.matmul(cp_ps, lhsT=ctxT_all,
                                 rhs=wproj_sb[:, nblk * 512:(nblk + 1) * 512],
                                 start=True, stop=True)
                nc.vector.tensor_copy(out_sb[:, nblk * 512:(nblk + 1) * 512], cp_ps)
            nc.sync.dma_start(out=out_part[:, :], in_=out_sb)

    nc.finalize()
    return nc


_CACHE = {}


def _prep_inputs(hidden_states, w_attn, b_attn, w_proj, key_cache, value_cache):
    hT = np.concatenate([np.ascontiguousarray(hidden_states.T),
                         np.ones((1, NUM_SEQS), np.float32)], axis=0)
    kc_flat = key_cache.reshape(TOT_SLOTS, NUM_HEADS, HEAD_DIM)
    vc_flat = value_cache.reshape(TOT_SLOTS, NUM_HEADS, HEAD_DIM)
    in_maps = []
    for c in range(N_CORES):
        h0 = c * HEADS_PER_CORE
        cols = []
        for part in range(3):  # q, k, v column blocks of w_attn
            base = part * EMBED + h0 * HEAD_DIM
            cols.append(np.arange(base, base + HD))
        cols = np.concatenate(cols)
        wqkv = np.concatenate([w_attn[:, cols], b_attn[cols][None, :]],
                              axis=0).astype(np.float32)
        wproj_c = np.ascontiguousarray(w_proj[h0 * HEAD_DIM:(h0 + HEADS_PER_CORE) * HEAD_DIM, :])
        kc_c = np.ascontiguousarray(kc_flat[:, h0:h0 + HEADS_PER_CORE, :]).reshape(TOT_SLOTS, HD)
        vc_c = np.ascontiguousarray(vc_flat[:, h0:h0 + HEADS_PER_CORE, :]).reshape(TOT_SLOTS, HD)
        in_maps.append({
            "hT": np.ascontiguousarray(hT),
            "wqkv": np.ascontiguousarray(wqkv),
            "wproj": wproj_c,
            "kc": kc_c,
            "vc": vc_c,
        })
    return in_maps


def kernel(hidden_states, w_attn, b_attn, w_proj, b_proj,
           key_cache, value_cache, block_tables, context_lens):
    from concourse.bass_utils import run_bass_kernel_spmd

    key = (context_lens.tobytes(), block_tables.tobytes())
    import hashlib
    key = hashlib.sha1(key[0] + key[1]).hexdigest()
    if key not in _CACHE:
        _CACHE[key] = _build_program(np.asarray(context_lens), np.asarray(block_tables))
    nc = _CACHE[key]

    in_maps = _prep_inputs(hidden_states, w_attn, b_attn, w_proj, key_cache, value_cache)
    res = run_bass_kernel_spmd(nc, in_maps, list(range(N_CORES)))
    out = np.zeros((NUM_SEQS, EMBED), np.float32)
    for r in res.results:
        out += r["out_part"]
    out += b_proj[None, :]
    return out

